# revision 3
# baseline (speedup 1.0000x reference)
"""Grouped BERT self-attention on 8 TRN2 NeuronCores.

Problem: G=4 groups, B=4 batch, L=512 seq, C=768 (12 heads x 64).
Sharding: the 16 (g, b) attention problems are embarrassingly parallel;
each core handles one group g = core//2 and two batches. Weights are
per-group so each core loads exactly one group's weights. No collectives.

Per-(g,b) on-chip dataflow (bf16 matmul inputs, fp32 accumulation):
  qT[d,l] = Wq[c,d].T @ hsT[c,l]   (weights in natural layout = lhsT)
  kT[d,l] = Wk[c,d].T @ hsT[c,l]
  v[m,d]  = hsT[c,m].T @ Wv[c,d]   (+bias, stored [m, head, 65] with a
                                    ones column per head for the softmax
                                    denominator)
  ST[m,l] = kT[d,m].T @ qT[d,l]    (heads paired on partitions 0:64/64:128
                                    -> concurrent PE row-tiles)
  E[m,l]  = exp(0.125*ST + mask[m])  (ACT, bias/scale folded, bf16 out)
  ctx[l,d]+denom = E[m,l].T @ v_aug[m,d+1]  (accumulate over m chunks)
  out[l,d] = ctx * (1/denom)       (per-partition tensor_scalar_mul)
"""

import numpy as np
import ml_dtypes

import concourse.bacc as bacc
import concourse.bass as bass
import concourse.tile as tile
import concourse.mybir as mybir
from concourse import bass_utils

# avoid FishPath artifact upload in the axon trace path
bass_utils.upload_artifacts = lambda tmpdir: tmpdir

G, B, L, C = 4, 4, 512, 768
NH, DH = 12, 64
NB = 2          # batches per core
CCH = C // 128  # 6 contraction chunks
LCH = L // 128  # 4 seq chunks
N_CORES = 8

BF16 = mybir.dt.bfloat16
F32 = mybir.dt.float32
NPBF16 = ml_dtypes.bfloat16

_COMPILED = None


def _build():
    nc = bacc.Bacc("TRN2", target_bir_lowering=False, debug=False)

    hst_d = nc.declare_dram_parameter("hst", [NB, 128, CCH, L], BF16, isOutput=False)
    wq_d = nc.declare_dram_parameter("wq", [128, CCH, C], BF16, isOutput=False)
    wk_d = nc.declare_dram_parameter("wk", [128, CCH, C], BF16, isOutput=False)
    wv_d = nc.declare_dram_parameter("wv", [128, CCH, C], BF16, isOutput=False)
    bq_d = nc.declare_dram_parameter("bq", [128, CCH], F32, isOutput=False)
    bk_d = nc.declare_dram_parameter("bk", [128, CCH], F32, isOutput=False)
    bvb_d = nc.declare_dram_parameter("bvb", [128, C], BF16, isOutput=False)
    mask_d = nc.declare_dram_parameter("mask", [NB, 128, LCH], F32, isOutput=False)
    out_d = nc.declare_dram_parameter("out", [NB, LCH, 128, C], F32, isOutput=True)

    with tile.TileContext(nc) as tc:
        with (
            tc.tile_pool(name="wpool", bufs=1) as wpool,
            tc.tile_pool(name="hpool", bufs=2) as hpool,
            tc.tile_pool(name="qkpool", bufs=2) as qkpool,
            tc.tile_pool(name="vpool", bufs=2 * LCH) as vpool,
            tc.tile_pool(name="epool", bufs=12) as epool,
            tc.tile_pool(name="cpool", bufs=2 * LCH) as cpool,
            tc.tile_pool(name="rpool", bufs=8) as rpool,
            tc.tile_pool(name="pbig", bufs=4, space=bass.MemorySpace.PSUM) as pbig,
            tc.tile_pool(name="ppv", bufs=4, space=bass.MemorySpace.PSUM) as ppv,
        ):
            wq = wpool.tile([128, CCH, C], BF16, tag="wq")
            wk = wpool.tile([128, CCH, C], BF16, tag="wk")
            wv = wpool.tile([128, CCH, C], BF16, tag="wv")
            bq = wpool.tile([128, CCH], F32, tag="bq")
            bk = wpool.tile([128, CCH], F32, tag="bk")
            bvb = wpool.tile([128, C], BF16, tag="bvb")
            nc.sync.dma_start(wq[:], wq_d[:])
            nc.sync.dma_start(wk[:], wk_d[:])
            nc.sync.dma_start(wv[:], wv_d[:])
            nc.sync.dma_start(bq[:], bq_d[:])
            nc.sync.dma_start(bk[:], bk_d[:])
            nc.sync.dma_start(bvb[:], bvb_d[:])

            for b in range(NB):
                hst = hpool.tile([128, CCH, L], BF16, tag="hst")
                msk = hpool.tile([128, LCH], F32, tag="mask")
                nc.sync.dma_start(hst[:], hst_d[b])
                nc.sync.dma_start(msk[:], mask_d[b])

                # ---- Q/K projections: qT/kT [d, l] with bias folded ----
                qt = qkpool.tile([128, CCH, L], BF16, tag="qt")
                kt = qkpool.tile([128, CCH, L], BF16, tag="kt")
                for w, bias, dst in ((wq, bq, qt), (wk, bk, kt)):
                    for j in range(CCH):
                        ps = pbig.tile([128, L], F32, tag="big")
                        for k in range(CCH):
                            nc.tensor.matmul(
                                ps[:],
                                w[:, k, 128 * j : 128 * (j + 1)],
                                hst[:, k, :],
                                start=(k == 0),
                                stop=(k == CCH - 1),
                            )
                        nc.scalar.activation(
                            dst[:, j, :], ps[:],
                            mybir.ActivationFunctionType.Identity,
                            bias=bias[:, j : j + 1], scale=1.0,
                        )

                # ---- V projection: token-major, bias added, ones col ----
                vt = [vpool.tile([128, NH, DH + 1], BF16, tag="v", name=f"v{t}") for t in range(LCH)]
                for t in range(LCH):
                    for half in range(2):
                        ncol = C // 2  # 384
                        ps = pbig.tile([128, ncol], F32, tag="big")
                        for k in range(CCH):
                            nc.tensor.matmul(
                                ps[:],
                                hst[:, k, 128 * t : 128 * (t + 1)],
                                wv[:, k, half * ncol : (half + 1) * ncol],
                                start=(k == 0),
                                stop=(k == CCH - 1),
                            )
                        nh2 = NH // 2
                        nc.vector.tensor_add(
                            vt[t][:, half * nh2 : (half + 1) * nh2, 0:DH],
                            ps[:].rearrange("p (h d) -> p h d", d=DH),
                            bvb[:, half * ncol : (half + 1) * ncol].rearrange(
                                "p (h d) -> p h d", d=DH
                            ),
                        )
                    nc.vector.memset(vt[t][:, :, DH : DH + 1], 1.0)

                # ---- attention, head pairs on partition halves ----
                ctx = [cpool.tile([128, C], F32, tag="ctx", name=f"ctx{lc}") for lc in range(LCH)]
                for hp in range(CCH):
                    e = [
                        [
                            epool.tile([128, L], BF16, tag="e", name=f"e{h2}_{mc}")
                            for mc in range(LCH)
                        ]
                        for h2 in range(2)
                    ]
                    for mc in range(LCH):
                        pss = [None, None]
                        for h2 in range(2):
                            pr = slice(64 * h2, 64 * (h2 + 1))
                            pss[h2] = pbig.tile(
                                [128, L], F32, tag="big", name=f"pss{h2}"
                            )
                            nc.tensor.matmul(
                                pss[h2][:],
                                kt[pr, hp, 128 * mc : 128 * (mc + 1)],
                                qt[pr, hp, :],
                            )
                        for h2 in range(2):
                            nc.scalar.activation(
                                e[h2][mc][:], pss[h2][:],
                                mybir.ActivationFunctionType.Exp,
                                bias=msk[:, mc : mc + 1], scale=0.125,
                            )
                    for h2 in range(2):
                        head = 2 * hp + h2
                        for lc in range(LCH):
                            pc = ppv.tile([128, DH + 1], F32, tag="pv")
                            for mc in range(LCH):
                                nc.tensor.matmul(
                                    pc[:],
                                    e[h2][mc][:, 128 * lc : 128 * (lc + 1)],
                                    vt[mc][:, head, :],
                                    start=(mc == 0),
                                    stop=(mc == LCH - 1),
                                )
                            rec = rpool.tile([128, 1], F32, tag="rec")
                            nc.vector.reciprocal(rec[:], pc[:, DH : DH + 1])
                            nc.vector.tensor_scalar_mul(
                                ctx[lc][:, DH * head : DH * (head + 1)],
                                pc[:, 0:DH],
                                rec[:],
                            )
                for lc in range(LCH):
                    nc.sync.dma_start(out_d[b, lc], ctx[lc][:])

    nc.compile()
    return nc


def _get_compiled():
    global _COMPILED
    if _COMPILED is None:
        _COMPILED = _build()
    return _COMPILED


def _prep_core(hs, mask, wq, wk, wv, bq, bk, bv, g, b0):
    hs_gb = np.ascontiguousarray(hs[g, b0 : b0 + NB])  # [2, L, C]
    # hst[b, p, j, l] = hs[g, b0+b, l, 128j+p]
    hst = np.ascontiguousarray(
        hs_gb.transpose(0, 2, 1).reshape(NB, CCH, 128, L).transpose(0, 2, 1, 3)
    ).astype(NPBF16)

    def wprep(w):
        # [p, j, d] = W[128j+p, d]
        return np.ascontiguousarray(
            w[g].reshape(CCH, 128, C).transpose(1, 0, 2)
        ).astype(NPBF16)

    bq_t = np.ascontiguousarray(bq[g, 0].reshape(CCH, 128).T).astype(np.float32)
    bk_t = np.ascontiguousarray(bk[g, 0].reshape(CCH, 128).T).astype(np.float32)
    bvb = np.ascontiguousarray(
        np.broadcast_to(bv[g, 0], (128, C))
    ).astype(NPBF16)
    # mask[b, p, mc] = mask[g, b0+b, 0, 0, 128mc+p]
    msk = np.ascontiguousarray(
        mask[g, b0 : b0 + NB, 0, 0].reshape(NB, LCH, 128).transpose(0, 2, 1)
    ).astype(np.float32)
    return {
        "hst": hst,
        "wq": wprep(wq),
        "wk": wprep(wk),
        "wv": wprep(wv),
        "bq": bq_t,
        "bk": bk_t,
        "bvb": bvb,
        "mask": msk,
    }


def kernel(
    hidden_states,
    attention_mask,
    query_weight,
    query_bias,
    key_weight,
    key_bias,
    value_weight,
    value_bias,
    _trace=False,
):
    hs = np.asarray(hidden_states, dtype=np.float32)
    mask = np.asarray(attention_mask, dtype=np.float32)
    wq = np.asarray(query_weight, dtype=np.float32)
    wk = np.asarray(key_weight, dtype=np.float32)
    wv = np.asarray(value_weight, dtype=np.float32)
    bq = np.asarray(query_bias, dtype=np.float32)
    bk = np.asarray(key_bias, dtype=np.float32)
    bv = np.asarray(value_bias, dtype=np.float32)

    nc = _get_compiled()
    in_maps = []
    for c in range(N_CORES):
        g, b0 = c // 2, NB * (c % 2)
        in_maps.append(_prep_core(hs, mask, wq, wk, wv, bq, bk, bv, g, b0))

    res = bass_utils.run_bass_kernel_spmd(
        nc, in_maps, core_ids=list(range(N_CORES)), trace=_trace
    )

    out = np.empty((G, B, L, C), dtype=np.float32)
    for c in range(N_CORES):
        g, b0 = c // 2, NB * (c % 2)
        o = res.results[c]["out"]  # [NB, LCH, 128, C]
        out[g, b0 : b0 + NB] = o.reshape(NB, L, C)
    if _trace:
        kernel.last_exec_time_ns = res.exec_time_ns
    return out
